# revision 50
# baseline (speedup 1.0000x reference)
"""HardClusterAssigner Trainium2 kernel.

Reference computation:
    x_emb = mean_b(einsum('bsv,hs->bvh', x, W) + b)   # [V, H]
    assignments = one_hot(argmin(-l2norm(x_emb) @ l2norm(centroids).T))

Key transformations:
  1. mean over B commutes with the linear contraction over S, the l2norm of
     the embedding is a positive per-row scale (argmin-invariant), and the
     1/B + bias fold in exactly:
         sim[v,c] = (sum_b x)[s,v] @ M[s,c] + bn[c],
         M = W.T @ cn.T,  bn = B * (b @ cn.T),  cn = l2norm(centroids)
     M/bn are x-independent and folded on the host (fp64); M ships as an
     exact fp16 hi+lo pair (~1e-7 relative), bn as an fp32 column appended
     to the stacked identity and added per-partition on the DVE.
  2. x streams as fp16 in [s, b, v] layout. Chunks 0..6 ride FULL-chunk
     tiles (1 MB, 8 KB descriptor rows — even the head stays off the
     ~341 GB/s 4 KB-descriptor-generation cap) and chunk 7 rides halves
     (short tail chain), so the whole stream is just 9 x-DMAs: barely
     past the 8-deep HWDGE in-flight window, every queue holds a deep
     backlog and the trigger pipeline can never starve the stream. All
     DMAs ride the single SP ring (the Act ring demonstrably disrupts
     the cadence). msb (2 KB rows) rides behind the first tile — slow
     sub-4KB descriptors only hurt when they are the head's only work —
     and idt (260 B rows) rides near the stream's END, since the tail
     needs it ~2us after the last byte. Each tile's b-reduction is a
     2-level halving add tree on the DVE (fp16 2x packed mode,
     contiguous slices).
  3. Mhi and Mlo sit side by side in one [128, 128] stationary, so the PE
     contracts each 512-column slab segment in a single matmul (hi-sims
     accumulate in PSUM partitions 0:C, lo-sims in C:2C) — half the
     matmul+ldweights traffic of a hi/lo double pump. The tail does ONE
     128-partition slab-reduce (same DVE cost as 64), adds bn via a
     [bn;0] per-partition column, and the transpose matmul's STACKED
     identity [I;I] folds hi+lo while flipping to [v, c] (exact: 1.0/0.0
     multiplies + fp32 PSUM adds); then row-max + is_equal one-hot.

Sharding: V is split across the 8 cores; every stage after the split is
core-local (no collectives). Per-core time is DMA-bound: ~8.7 MB/core
streamed at the HBM roofline, with the DVE trees (~17us) and all PE work
(~12us) hidden under the stream.
"""

import sys

for _p in ("/opt/trn_rl_repo",):
    if _p not in sys.path:
        sys.path.append(_p)

from contextlib import ExitStack

import numpy as np

import concourse.bacc as bacc
import concourse.bass as bass
import concourse.mybir as mybir
from concourse import tile
from concourse.bass_utils import run_bass_kernel_spmd

B, S, V, H, C = 64, 1024, 512, 512, 64
NCORES = 8
VL = V // NCORES  # 64 V-columns per core
P = 128
ST = S // P  # 8 s-chunks
ROW = B * VL  # 4096 fp16 elems per s-row
MMW = 512  # matmul segment width / slab width (8 slabs of VL)
F16 = mybir.dt.float16
F32 = mybir.dt.float32

# x tile plan: (t, col_lo, col_hi); chunk 7 as halves (short tail), rest
# full — the head tile's 8 KB rows keep even the ramp off the ~341 GB/s
# 4 KB-descriptor-generation cap
TILES = (
    [(t, 0, ROW) for t in range(7)]
    + [(7, 0, ROW // 2), (7, ROW // 2, ROW)]
)
DEPTH = 2  # all trees 2-level (verified argmax-exact)

_NC_CACHE = None


def build_bass() -> bass.Bass:
    nc = bacc.Bacc("TRN2", target_bir_lowering=False)

    xs = nc.declare_dram_parameter("xs", [S, ROW], F16, isOutput=False)
    # stationary: per s-chunk a [P, 2C] block = [Mhi_t | Mlo_t]
    mm = nc.declare_dram_parameter("mm", [P, ST * 2 * C], F16, isOutput=False)
    # [I64; I64] stacked identity | [bn; 0] column
    c32 = nc.declare_dram_parameter("c32", [P, C + 1], F32, isOutput=False)
    out = nc.declare_dram_parameter("out", [VL, C], F32, isOutput=True)

    nmm = sum(((c1 - c0) >> DEPTH) // MMW for _, c0, c1 in TILES)

    with tile.TileContext(nc) as tc, ExitStack() as ctx:
        consts = ctx.enter_context(tc.tile_pool(name="consts", bufs=1))
        xpool = ctx.enter_context(tc.tile_pool(name="xp", bufs=1))
        spool = ctx.enter_context(tc.tile_pool(name="small", bufs=1))
        psp = ctx.enter_context(tc.tile_pool(name="psp", bufs=1, space="PSUM"))

        msb = consts.tile([P, ST * 2 * C], F16)
        idt = consts.tile([P, C + 1], F32)

        # [c | c+64] rows hold the hi / lo partial sims
        psC = psp.tile([2 * C, MMW], F32, tag="psC")
        psT = psp.tile([VL, C], F32, tag="psT")

        xs_r = xs.rearrange("(t p) f -> t p f", p=P)
        seen = 0
        for ti, (t, c0, c1) in enumerate(TILES):
            width = c1 - c0
            xv = xpool.tile([P, width], F16, tag=f"x{ti}", name=f"xv{ti}")
            nc.sync.dma_start(out=xv[:], in_=xs_r[t][:, c0:c1])
            if ti == 0:
                # msb lands behind the first tile (its 2KB rows halve the
                # per-queue descriptor rate, which only matters when they
                # are the head's ONLY work); the PE needs M no sooner than
                # the first tree finish, ~2us after msb lands. Program
                # order (before tile 0's matmul) carries the dependency.
                nc.sync.dma_start(out=msb[:], in_=mm[:])
            if ti == len(TILES) - 2:
                # idt (128 slow 260B descriptors, needed only by the tail
                # ~2us after the last byte) rides near the stream's end so
                # its sub-KB rows never perturb the ramp
                nc.sync.dma_start(out=idt[:], in_=c32[:])
            # halving add tree over contiguous column blocks (fp16 2x mode)
            nb = width
            for _ in range(DEPTH):
                hb = nb // 2
                nc.vector.tensor_tensor(
                    xv[:, 0:hb], xv[:, 0:hb], xv[:, hb:nb],
                    op=mybir.AluOpType.add,
                )
                nb = hb
            # slab contraction, [Mhi|Mlo] stationary; segments overlay the
            # same PSUM columns (sums just accumulate)
            for g in range(0, nb, MMW):
                nc.tensor.matmul(
                    psC[:],
                    msb[:, t * 2 * C : (t + 1) * 2 * C],
                    xv[:, g : g + MMW],
                    start=(seen == 0),
                    stop=(seen == nmm - 1),
                )
                seen += 1

        # --- tail: one 128-partition slab-reduce, bn add, hi+lo-folding
        # transpose, row-max + is_equal one-hot, store -----------------------
        sC2 = spool.tile([2 * C, VL], F32)
        nc.vector.tensor_reduce(
            sC2[:],
            psC[:].rearrange("c (s v) -> c v s", s=MMW // VL),
            axis=mybir.AxisListType.X,
            op=mybir.AluOpType.add,
        )
        nc.vector.tensor_scalar(
            sC2[:], sC2[:], idt[:, C : C + 1], None, op0=mybir.AluOpType.add
        )
        nc.tensor.matmul(psT[:], sC2[:], idt[:, 0:C], start=True, stop=True)

        mx = spool.tile([VL, 1], F32)
        nc.vector.tensor_reduce(
            mx[:], psT[:], axis=mybir.AxisListType.X, op=mybir.AluOpType.max
        )
        oh = spool.tile([VL, C], F32)
        nc.vector.tensor_scalar(
            oh[:], psT[:], mx[:], None, op0=mybir.AluOpType.is_equal
        )
        nc.sync.dma_start(out=out[:], in_=oh[:])

    nc.compile()
    return nc


def _get_nc() -> bass.Bass:
    global _NC_CACHE
    if _NC_CACHE is None:
        _NC_CACHE = build_bass()
    return _NC_CACHE


def make_in_maps(x, W, b, centroids):
    x = np.asarray(x, dtype=np.float32)
    W = np.asarray(W, dtype=np.float32)
    b = np.asarray(b, dtype=np.float32)
    centroids = np.asarray(centroids, dtype=np.float32)

    # x-independent folds, in float64, shipped as exact fp16 hi+lo pairs
    cn = centroids.astype(np.float64)
    cn /= np.linalg.norm(cn, axis=1, keepdims=True)
    M = W.astype(np.float64).T @ cn.T  # [S, C]
    bn = np.float64(B) * (b.astype(np.float64) @ cn.T)  # [C]

    Mhi = M.astype(np.float16)
    Mlo = (M - Mhi.astype(np.float64)).astype(np.float16)
    # [P, ST, 2C] with [Mhi_t | Mlo_t] per chunk
    mhost = np.empty((P, ST, 2 * C), np.float16)
    mhost[:, :, 0:C] = Mhi.reshape(ST, P, C).transpose(1, 0, 2)
    mhost[:, :, C : 2 * C] = Mlo.reshape(ST, P, C).transpose(1, 0, 2)
    mhost = np.ascontiguousarray(mhost).reshape(P, ST * 2 * C)

    # [I64; I64] stacked identity | [bn; 0] column
    c32host = np.zeros((P, C + 1), np.float32)
    eye = np.eye(C, dtype=np.float32)
    c32host[0:C, 0:C] = eye
    c32host[C : 2 * C, 0:C] = eye
    c32host[0:C, C] = bn.astype(np.float32)

    # Host layout [B,S,V] -> [S, B, VL] per core, in fp16 (cast first so the
    # transpose moves half the bytes). One pass to [S, B, V] (contiguous 1KB
    # runs), then a contiguous per-core V-slice.
    x16 = x.astype(np.float16)
    xsb = np.ascontiguousarray(x16.transpose(1, 0, 2))  # [S, B, V]
    in_maps = []
    for i in range(NCORES):
        xs_i = np.ascontiguousarray(
            xsb[:, :, i * VL : (i + 1) * VL]
        ).reshape(S, ROW)
        in_maps.append({"xs": xs_i, "mm": mhost, "c32": c32host})
    return in_maps


def run(inputs: dict, trace: bool = False):
    """Run on the 8 NeuronCores; returns (full_output, BassKernelResults)."""
    nc = _get_nc()
    in_maps = make_in_maps(**inputs)
    res = run_bass_kernel_spmd(nc, in_maps, list(range(NCORES)), trace=trace)
    full = np.concatenate([r["out"] for r in res.results], axis=0)
    return full, res


def kernel(x, W, b, centroids) -> np.ndarray:
    full, _ = run({"x": x, "W": W, "b": b, "centroids": centroids})
    return full


# revision 52
# speedup vs baseline: 1.0319x; 1.0319x over previous
"""HardClusterAssigner Trainium2 kernel.

Reference computation:
    x_emb = mean_b(einsum('bsv,hs->bvh', x, W) + b)   # [V, H]
    assignments = one_hot(argmin(-l2norm(x_emb) @ l2norm(centroids).T))

Key transformations:
  1. mean over B commutes with the linear contraction over S, the l2norm of
     the embedding is a positive per-row scale (argmin-invariant), and the
     1/B + bias fold in exactly:
         sim[v,c] = (sum_b x)[s,v] @ M[s,c] + bn[c],
         M = W.T @ cn.T,  bn = B * (b @ cn.T),  cn = l2norm(centroids)
     M/bn are x-independent and folded on the host (fp64); M ships as an
     exact fp16 hi+lo pair (~1e-7 relative), bn as an fp32 column appended
     to the stacked identity and added per-partition on the DVE.
  2. x streams as fp16 in [s, b, v] layout. Chunks 0..6 ride FULL-chunk
     tiles (1 MB, 8 KB descriptor rows — even the head stays off the
     ~341 GB/s 4 KB-descriptor-generation cap) and chunk 7 rides halves
     (short tail chain), so the whole stream is just 9 x-DMAs: barely
     past the 8-deep HWDGE in-flight window, every queue holds a deep
     backlog and the trigger pipeline can never starve the stream. All
     DMAs ride the single SP ring (the Act ring demonstrably disrupts
     the cadence). msb (2 KB rows) rides behind the first tile — slow
     sub-4KB descriptors only hurt when they are the head's only work —
     and idt (260 B rows) rides near the stream's END, since the tail
     needs it ~2us after the last byte. Each tile's b-reduction is a
     2-level halving add tree on the DVE (fp16 2x packed mode,
     contiguous slices).
  3. Mhi and Mlo sit side by side in one [128, 128] stationary, so the PE
     contracts each 512-column slab segment in a single matmul (hi-sims
     accumulate in PSUM partitions 0:C, lo-sims in C:2C) — half the
     matmul+ldweights traffic of a hi/lo double pump. The tail does ONE
     128-partition slab-reduce (same DVE cost as 64), adds bn via a
     [bn;0] per-partition column, and the transpose matmul's STACKED
     identity [I;I] folds hi+lo while flipping to [v, c] (exact: 1.0/0.0
     multiplies + fp32 PSUM adds); then row-max + is_equal one-hot.

Sharding: V is split across the 8 cores; every stage after the split is
core-local (no collectives). Per-core time is DMA-bound: ~8.7 MB/core
streamed at the HBM roofline, with the DVE trees (~17us) and all PE work
(~12us) hidden under the stream.
"""

import sys

for _p in ("/opt/trn_rl_repo",):
    if _p not in sys.path:
        sys.path.append(_p)

from contextlib import ExitStack

import numpy as np

import concourse.bacc as bacc
import concourse.bass as bass
import concourse.mybir as mybir
from concourse import tile
from concourse.bass_utils import run_bass_kernel_spmd

B, S, V, H, C = 64, 1024, 512, 512, 64
NCORES = 8
VL = V // NCORES  # 64 V-columns per core
P = 128
ST = S // P  # 8 s-chunks
ROW = B * VL  # 4096 fp16 elems per s-row
MMW = 512  # matmul segment width / slab width (8 slabs of VL)
F16 = mybir.dt.float16
F32 = mybir.dt.float32

# x tile plan: (t, col_lo, col_hi); chunk 7 as halves (short tail), rest
# full — the head tile's 8 KB rows keep even the ramp off the ~341 GB/s
# 4 KB-descriptor-generation cap
TILES = (
    [(t, 0, ROW) for t in range(7)]
    + [(7, 0, ROW // 2), (7, ROW // 2, ROW)]
)
DEPTH = 2  # all trees 2-level (verified argmax-exact)

_NC_CACHE = None


def build_bass() -> bass.Bass:
    nc = bacc.Bacc("TRN2", target_bir_lowering=False)

    xs = nc.declare_dram_parameter("xs", [S, ROW], F16, isOutput=False)
    # stationary: per s-chunk a [P, 2C] block = [Mhi_t | Mlo_t]
    mm = nc.declare_dram_parameter("mm", [P, ST * 2 * C], F16, isOutput=False)
    # [I64; I64] stacked identity | [bn; 0] column
    c32 = nc.declare_dram_parameter("c32", [P, C + 1], F32, isOutput=False)
    out = nc.declare_dram_parameter("out", [VL, C], F32, isOutput=True)

    nmm = sum(((c1 - c0) >> DEPTH) // MMW for _, c0, c1 in TILES)

    with tile.TileContext(nc) as tc, ExitStack() as ctx:
        consts = ctx.enter_context(tc.tile_pool(name="consts", bufs=1))
        xpool = ctx.enter_context(tc.tile_pool(name="xp", bufs=1))
        spool = ctx.enter_context(tc.tile_pool(name="small", bufs=1))
        psp = ctx.enter_context(tc.tile_pool(name="psp", bufs=1, space="PSUM"))

        msb = consts.tile([P, ST * 2 * C], F16)
        idt = consts.tile([P, C + 1], F32)

        # [c | c+64] rows hold the hi / lo partial sims
        psC = psp.tile([2 * C, MMW], F32, tag="psC")
        psT = psp.tile([VL, C], F32, tag="psT")

        xs_r = xs.rearrange("(t p) f -> t p f", p=P)
        seen = 0
        for ti, (t, c0, c1) in enumerate(TILES):
            width = c1 - c0
            xv = xpool.tile([P, width], F16, tag=f"x{ti}", name=f"xv{ti}")
            nc.sync.dma_start(out=xv[:], in_=xs_r[t][:, c0:c1])
            if ti == 0:
                # msb lands behind the first tile (its 2KB rows halve the
                # per-queue descriptor rate, which only matters when they
                # are the head's ONLY work); the PE needs M no sooner than
                # the first tree finish, ~2us after msb lands. Program
                # order (before tile 0's matmul) carries the dependency.
                nc.sync.dma_start(out=msb[:], in_=mm[:])
            # halving add tree over contiguous column blocks (fp16 2x mode)
            nb = width
            for _ in range(DEPTH):
                hb = nb // 2
                nc.vector.tensor_tensor(
                    xv[:, 0:hb], xv[:, 0:hb], xv[:, hb:nb],
                    op=mybir.AluOpType.add,
                )
                nb = hb
            # slab contraction, [Mhi|Mlo] stationary; segments overlay the
            # same PSUM columns (sums just accumulate)
            for g in range(0, nb, MMW):
                nc.tensor.matmul(
                    psC[:],
                    msb[:, t * 2 * C : (t + 1) * 2 * C],
                    xv[:, g : g + MMW],
                    start=(seen == 0),
                    stop=(seen == nmm - 1),
                )
                seen += 1

        # idt rides AFTER the last x tile: queue service is ~155ns per
        # descriptor regardless of size, so its 128 sub-KB descriptors cost
        # ~1.2us of queue time wherever they sit — behind t7h1 that time
        # comes off the critical last x byte, and idt still lands ~1.5us
        # before the bn add needs it
        nc.sync.dma_start(out=idt[:], in_=c32[:])

        # --- tail: one 128-partition slab-reduce, bn add, hi+lo-folding
        # transpose, row-max + is_equal one-hot, store -----------------------
        sC2 = spool.tile([2 * C, VL], F32)
        nc.vector.tensor_reduce(
            sC2[:],
            psC[:].rearrange("c (s v) -> c v s", s=MMW // VL),
            axis=mybir.AxisListType.X,
            op=mybir.AluOpType.add,
        )
        nc.vector.tensor_scalar(
            sC2[:], sC2[:], idt[:, C : C + 1], None, op0=mybir.AluOpType.add
        )
        nc.tensor.matmul(psT[:], sC2[:], idt[:, 0:C], start=True, stop=True)

        mx = spool.tile([VL, 1], F32)
        nc.vector.tensor_reduce(
            mx[:], psT[:], axis=mybir.AxisListType.X, op=mybir.AluOpType.max
        )
        oh = spool.tile([VL, C], F32)
        nc.vector.tensor_scalar(
            oh[:], psT[:], mx[:], None, op0=mybir.AluOpType.is_equal
        )
        nc.sync.dma_start(out=out[:], in_=oh[:])

    nc.compile()
    return nc


def _get_nc() -> bass.Bass:
    global _NC_CACHE
    if _NC_CACHE is None:
        _NC_CACHE = build_bass()
    return _NC_CACHE


def make_in_maps(x, W, b, centroids):
    x = np.asarray(x, dtype=np.float32)
    W = np.asarray(W, dtype=np.float32)
    b = np.asarray(b, dtype=np.float32)
    centroids = np.asarray(centroids, dtype=np.float32)

    # x-independent folds, in float64, shipped as exact fp16 hi+lo pairs
    cn = centroids.astype(np.float64)
    cn /= np.linalg.norm(cn, axis=1, keepdims=True)
    M = W.astype(np.float64).T @ cn.T  # [S, C]
    bn = np.float64(B) * (b.astype(np.float64) @ cn.T)  # [C]

    Mhi = M.astype(np.float16)
    Mlo = (M - Mhi.astype(np.float64)).astype(np.float16)
    # [P, ST, 2C] with [Mhi_t | Mlo_t] per chunk
    mhost = np.empty((P, ST, 2 * C), np.float16)
    mhost[:, :, 0:C] = Mhi.reshape(ST, P, C).transpose(1, 0, 2)
    mhost[:, :, C : 2 * C] = Mlo.reshape(ST, P, C).transpose(1, 0, 2)
    mhost = np.ascontiguousarray(mhost).reshape(P, ST * 2 * C)

    # [I64; I64] stacked identity | [bn; 0] column
    c32host = np.zeros((P, C + 1), np.float32)
    eye = np.eye(C, dtype=np.float32)
    c32host[0:C, 0:C] = eye
    c32host[C : 2 * C, 0:C] = eye
    c32host[0:C, C] = bn.astype(np.float32)

    # Host layout [B,S,V] -> [S, B, VL] per core, in fp16 (cast first so the
    # transpose moves half the bytes). One pass to [S, B, V] (contiguous 1KB
    # runs), then a contiguous per-core V-slice.
    x16 = x.astype(np.float16)
    xsb = np.ascontiguousarray(x16.transpose(1, 0, 2))  # [S, B, V]
    in_maps = []
    for i in range(NCORES):
        xs_i = np.ascontiguousarray(
            xsb[:, :, i * VL : (i + 1) * VL]
        ).reshape(S, ROW)
        in_maps.append({"xs": xs_i, "mm": mhost, "c32": c32host})
    return in_maps


def run(inputs: dict, trace: bool = False):
    """Run on the 8 NeuronCores; returns (full_output, BassKernelResults)."""
    nc = _get_nc()
    in_maps = make_in_maps(**inputs)
    res = run_bass_kernel_spmd(nc, in_maps, list(range(NCORES)), trace=trace)
    full = np.concatenate([r["out"] for r in res.results], axis=0)
    return full, res


def kernel(x, W, b, centroids) -> np.ndarray:
    full, _ = run({"x": x, "W": W, "b": b, "centroids": centroids})
    return full
